# revision 22
# baseline (speedup 1.0000x reference)
"""CBOW negative-sampling loss kernel for 8 TRN2 NeuronCores.

Strategy: data-parallel over batch (32 batch elems / core). The one-hot
inputs (1 GB total) are streamed once through SBUF; a single fused DVE
tensor_tensor_reduce per [128, F] chunk multiplies by a local iota and
reduce-adds, recovering each row's one-hot position in one pass. A tiny
cleanup converts per-chunk partial indices to global rows of W = [V; U],
gathered via indirect DMA (608 rows x 512 B / core). The remaining math
([608, 128] embeddings -> 32 per-batch losses) runs on PE (selection-matrix
matmuls for the context mean, vo broadcast, and k-sums) and ACT (Softplus).
Host sums the 8x32 per-batch losses and divides by 256.
"""

import numpy as np

VOC, DIM = 50000, 128
B, CTX, K = 256, 8, 10
NCORES = 8
BL = B // NCORES            # 32 batch elements per core
F = 5000                    # vocab chunk (free dim) per DVE op
NCH = VOC // F              # 10 chunks
NVI = BL * CTX              # 256 vi rows
NNEG = BL * K               # 320 neg rows
ROWS = BL + NVI + NNEG      # 608 one-hot rows per core
NG = (ROWS + 127) // 128    # 5 row groups
PG = [min(128, ROWS - g * 128) for g in range(NG)]  # [128,128,128,128,96]

_F32 = None  # set lazily (mybir.dt.float32)


def _row_kind(r):
    """Row r of the per-core row space -> (kind, batch, sub)."""
    if r < BL:
        return ("vo", r, 0)
    if r < BL + NVI:
        f = r - BL
        return ("vi", f // CTX, f % CTX)
    n = r - BL - NVI
    return ("neg", n // K, n % K)


def _make_constants():
    """Host-side constant tables shared by all cores."""
    bases = np.zeros((NG * 128, NCH), np.float32)
    for g in range(NG):
        for p in range(PG[g]):
            kind, _, _ = _row_kind(g * 128 + p)
            off = 0.0 if kind == "vo" else float(VOC)
            for c in range(NCH):
                bases[g * 128 + p, c] = c * F - 1 + off

    svi = np.zeros((3 * 128, BL), np.float32)     # groups 0,1,2: vi mean (1/CTX)
    for gi, g in enumerate((0, 1, 2)):
        for p in range(PG[g]):
            kind, b, _ = _row_kind(g * 128 + p)
            if kind == "vi":
                svi[gi * 128 + p, b] = 1.0 / CTX

    bvo = np.zeros((3 * BL, 128), np.float32)     # groups 2,3,4: vo broadcast
    nsel = np.zeros((3 * 128, BL), np.float32)    # groups 2,3,4: k-sum
    for gi, g in enumerate((2, 3, 4)):
        for m in range(PG[g]):
            kind, b, _ = _row_kind(g * 128 + m)
            if kind == "neg":
                bvo[gi * BL + b, m] = 1.0
                nsel[gi * 128 + m, b] = 1.0
    return bases, svi, bvo, nsel


def _split_excess_waits(nc, max_waits=1):
    """walrus CoreV3 in this container rejects instructions carrying more
    than one sync-wait; hoist extras onto standalone NoOps placed before."""
    import concourse.mybir as mybir

    uid = 0
    for f in nc.m.functions:
        for blk in f.blocks:
            insts = blk.instructions
            new_insts = []
            changed = False
            for inst in insts:
                si = inst.sync_info
                if si is not None and si.on_wait and len(si.on_wait) > max_waits:
                    waits = list(si.on_wait)
                    extra, keep = waits[:-max_waits], waits[-max_waits:]
                    for i in range(0, len(extra), max_waits):
                        nop = mybir.InstNoOp(
                            name=f"I-waitsplit-{uid}",
                            sync_info=mybir.SyncInfo(
                                on_wait=extra[i : i + max_waits], on_update=[]
                            ),
                            bass_nofuse=True,
                            engine=inst.engine,
                        )
                        uid += 1
                        new_insts.append(nop)
                    inst.sync_info = mybir.SyncInfo(
                        on_wait=keep, on_update=list(si.on_update or [])
                    )
                    changed = True
                new_insts.append(inst)
            if changed:
                blk.instructions = new_insts


def _group_srcs(vo_h, vi_h, neg_h, g, c):
    """DMA sources filling row-group g's [128, F] tile for vocab chunk c."""
    sl = slice(c * F, (c + 1) * F)
    if g == 0:
        return [(0, 32, vo_h[0:32, sl]), (32, 128, vi_h[0:96, sl])]
    if g == 1:
        return [(0, 128, vi_h[96:224, sl])]
    if g == 2:
        return [(0, 32, vi_h[224:256, sl]), (32, 128, neg_h[0:96, sl])]
    if g == 3:
        return [(0, 128, neg_h[96:224, sl])]
    return [(0, 96, neg_h[224:320, sl])]


def _build_nc(oh_bufs=4, scr_bufs=2, reps=1, dma_eng="gpsimd"):
    from contextlib import ExitStack

    import concourse.bass as bass
    import concourse.mybir as mybir
    import concourse.tile as tile

    f32 = mybir.dt.float32
    i32 = mybir.dt.int32
    Alu = mybir.AluOpType
    Act = mybir.ActivationFunctionType

    nc = bass.Bass("TRN2", target_bir_lowering=False, debug=False)

    vo_h = nc.declare_dram_parameter("vo", [BL, VOC], f32, isOutput=False)
    vi_h = nc.declare_dram_parameter("vi", [NVI, VOC], f32, isOutput=False)
    neg_h = nc.declare_dram_parameter("neg", [NNEG, VOC], f32, isOutput=False)
    w_h = nc.declare_dram_parameter("w", [2 * VOC, DIM], f32, isOutput=False)
    bases_h = nc.declare_dram_parameter("bases", [NG * 128, NCH], f32, isOutput=False)
    svi_h = nc.declare_dram_parameter("svi", [3 * 128, BL], f32, isOutput=False)
    bvo_h = nc.declare_dram_parameter("bvo", [3 * BL, 128], f32, isOutput=False)
    nsel_h = nc.declare_dram_parameter("nsel", [3 * 128, BL], f32, isOutput=False)
    out_h = nc.declare_dram_parameter("out", [BL, 1], f32, isOutput=True)

    with ExitStack() as ctx:
        tc = ctx.enter_context(tile.TileContext(nc))
        cpool = ctx.enter_context(tc.tile_pool(name="consts", bufs=1))
        ohp = ctx.enter_context(tc.tile_pool(name="oh", bufs=oh_bufs))
        scrp = ctx.enter_context(tc.tile_pool(name="scr", bufs=scr_bufs))
        accp = ctx.enter_context(tc.tile_pool(name="acc", bufs=1))
        small = ctx.enter_context(tc.tile_pool(name="small", bufs=1))
        psum = ctx.enter_context(tc.tile_pool(name="psum", bufs=1, space="PSUM"))

        iota_t = cpool.tile([128, F], f32, tag="iota")
        nc.gpsimd.iota(
            iota_t[:],
            pattern=[[1, F]],
            base=1,
            channel_multiplier=0,
            allow_small_or_imprecise_dtypes=True,
        )

        # touch Exp/Ln so their shared ACT table loads during streaming,
        # not on the tail critical path
        warm = cpool.tile([1, 1], f32, tag="warm")
        nc.vector.tensor_copy(warm[:, :], iota_t[0:1, 0:1])
        nc.scalar.activation(warm[:, :], warm[:, :], Act.Exp)
        nc.scalar.activation(warm[:, :], warm[:, :], Act.Ln, bias=1.0)

        bases_t = []
        for g in range(NG):
            t = cpool.tile([128, NCH], f32, tag=f"base{g}")
            nc.gpsimd.dma_start(
                out=t[: PG[g], :], in_=bases_h[g * 128 : g * 128 + PG[g], :]
            )
            bases_t.append(t)
        svi_t = []
        for gi in range(3):
            t = cpool.tile([128, BL], f32, tag=f"svi{gi}")
            nc.gpsimd.dma_start(out=t[:, :], in_=svi_h[gi * 128 : (gi + 1) * 128, :])
            svi_t.append(t)
        bvo_t = []
        nsel_t = []
        for gi, g in enumerate((2, 3, 4)):
            t = cpool.tile([BL, 128], f32, tag=f"bvo{gi}")
            nc.gpsimd.dma_start(out=t[:, :], in_=bvo_h[gi * BL : (gi + 1) * BL, :])
            bvo_t.append(t)
            t = cpool.tile([128, BL], f32, tag=f"nsel{gi}")
            nc.gpsimd.dma_start(
                out=t[: PG[g], :], in_=nsel_h[gi * 128 : gi * 128 + PG[g], :]
            )
            nsel_t.append(t)

        # ---- heavy phase: stream one-hot, fused mul+reduce against iota ----
        for rep in range(reps):
            _emit_body(
                nc, tc, mybir, bass, ohp, scrp, accp, small, psum,
                iota_t, bases_t, svi_t, bvo_t, nsel_t,
                vo_h, vi_h, neg_h, w_h, out_h, rep, dma_eng,
            )

    _split_excess_waits(nc)
    return nc


def _emit_body(
    nc, tc, mybir, bass, ohp, scrp, accp, small, psum,
    iota_t, bases_t, svi_t, bvo_t, nsel_t,
    vo_h, vi_h, neg_h, w_h, out_h, rep, dma_eng="gpsimd",
):
    f32 = mybir.dt.float32
    i32 = mybir.dt.int32
    Alu = mybir.AluOpType
    Act = mybir.ActivationFunctionType
    if dma_eng == "alt":
        dmas = [nc.sync, nc.gpsimd]
    elif dma_eng == "alt2hw":
        dmas = [nc.sync, nc.scalar]
    else:
        dmas = [getattr(nc, dma_eng)]
    if True:
        acc_t = [
            accp.tile([128, NCH], f32, name=f"acc{g}", tag=f"acc{g}")
            for g in range(NG)
        ]
        emb_t = []
        for g in range(NG):
            p = PG[g]
            for c in range(NCH):
                oh = ohp.tile([128, F], f32, tag="oh")
                for lo, hi, src in _group_srcs(vo_h, vi_h, neg_h, g, c):
                    dmas[(g * NCH + c) % len(dmas)].dma_start(
                        out=oh[lo:hi, :], in_=src
                    )
                scr = scrp.tile([128, F], f32, tag="scr")
                nc.vector.tensor_mul(scr[:p, :], oh[:p, :], iota_t[:p, :])
                nc.scalar.activation(
                    out=scr[:p, :],
                    in_=scr[:p, :],
                    func=Act.Identity,
                    accum_out=acc_t[g][:p, c : c + 1],
                )

            # ---- per-group cleanup: partial sums -> global W row index ----
            hit = small.tile([128, NCH], f32, tag=f"hit{g}")
            nc.vector.tensor_scalar_min(hit[:p, :], acc_t[g][:p, :], 1.0)
            t1 = small.tile([128, NCH], f32, tag=f"t1{g}")
            nc.vector.tensor_mul(t1[:p, :], hit[:p, :], bases_t[g][:p, :])
            t2 = small.tile([128, NCH], f32, tag=f"t2{g}")
            nc.vector.tensor_add(t2[:p, :], acc_t[g][:p, :], t1[:p, :])
            idxf = small.tile([128, 1], f32, tag=f"idxf{g}")
            nc.vector.tensor_reduce(
                idxf[:p, :], t2[:p, :], axis=mybir.AxisListType.X, op=Alu.add
            )
            # clamp in-bounds (safety; also keeps rep>0 timing runs legal)
            nc.vector.tensor_scalar_min(idxf[:p, :], idxf[:p, :], float(2 * VOC - 1))
            idxi = small.tile([128, 1], i32, tag=f"idxi{g}")
            nc.vector.tensor_copy(idxi[:p, :], idxf[:p, :])

            emb = small.tile([128, DIM], f32, tag=f"emb{g}")
            nc.gpsimd.indirect_dma_start(
                out=emb[:p, :],
                out_offset=None,
                in_=w_h[:, :],
                in_offset=bass.IndirectOffsetOnAxis(ap=idxi[:p, 0:1], axis=0),
            )
            emb_t.append(emb)

        # ---- tail: vi mean, dots, softplus, k-sums -> per-batch loss ----
        vi_ps = psum.tile([BL, DIM], f32, tag="vi_ps")
        for gi, g in enumerate((0, 1, 2)):
            nc.tensor.matmul(
                out=vi_ps[:, :],
                lhsT=svi_t[gi][: PG[g], :],
                rhs=emb_t[g][: PG[g], :],
                start=(gi == 0),
                stop=(gi == 2),
            )

        vo_emb = emb_t[0][0:BL, :]
        scrd = small.tile([BL, DIM], f32, tag="scrd")
        dotvv = small.tile([BL, 1], f32, tag="dotvv")
        nc.vector.tensor_mul(scrd[:, :], vo_emb, vi_ps[:, :])
        nc.vector.tensor_reduce(
            dotvv[:, :], scrd[:, :], axis=mybir.AxisListType.X, op=Alu.add
        )
        # softplus(x) = ln(1 + exp(x)); ln & exp share one ACT table set here
        nl_e = small.tile([BL, 1], f32, tag="nl_e")
        nc.scalar.activation(nl_e[:, :], dotvv[:, :], Act.Exp, scale=-1.0)
        nl = small.tile([BL, 1], f32, tag="nl")
        nc.scalar.activation(nl[:, :], nl_e[:, :], Act.Ln, bias=1.0)

        right_ps = psum.tile([BL, 1], f32, tag="right_ps")
        for gi, g in enumerate((2, 3, 4)):
            p = PG[g]
            bc = psum.tile([128, DIM], f32, tag=f"bc{gi}")
            nc.tensor.matmul(
                out=bc[:p, :],
                lhsT=bvo_t[gi][:, :p],
                rhs=vo_emb,
                start=True,
                stop=True,
            )
            scr2 = small.tile([128, DIM], f32, tag=f"scr2{gi}")
            nd = small.tile([128, 1], f32, tag=f"nd{gi}")
            nc.vector.tensor_mul(scr2[:p, :], emb_t[g][:p, :], bc[:p, :])
            nc.vector.tensor_reduce(
                nd[:p, :], scr2[:p, :], axis=mybir.AxisListType.X, op=Alu.add
            )
            sp_e = small.tile([128, 1], f32, tag=f"sp_e{gi}")
            nc.scalar.activation(sp_e[:p, :], nd[:p, :], Act.Exp)
            sp = small.tile([128, 1], f32, tag=f"sp{gi}")
            nc.scalar.activation(sp[:p, :], sp_e[:p, :], Act.Ln, bias=1.0)
            nc.tensor.matmul(
                out=right_ps[:, :],
                lhsT=nsel_t[gi][:p, :],
                rhs=sp[:p, :],
                start=(gi == 0),
                stop=(gi == 2),
            )

        loss = small.tile([BL, 1], f32, tag="loss")
        nc.vector.tensor_add(loss[:, :], nl[:, :], right_ps[:, :])
        nc.gpsimd.dma_start(out=out_h[:, :], in_=loss[:, :])


_BUILT = {}


def _get_nc(key=(4, 2, 1, "alt")):
    if key not in _BUILT:
        _BUILT[key] = _build_nc(*key)
    return _BUILT[key]


def _make_in_maps(vo, vi, neg_samples, V, U):
    vo = np.ascontiguousarray(vo, np.float32)
    vi = np.ascontiguousarray(vi, np.float32).reshape(B, CTX, VOC)
    neg = np.ascontiguousarray(neg_samples, np.float32).reshape(B, K, VOC)
    w = np.ascontiguousarray(np.concatenate([V, U], axis=0), np.float32)
    bases, svi, bvo, nsel = _make_constants()
    in_maps = []
    for core in range(NCORES):
        b0 = core * BL
        in_maps.append(
            {
                "vo": vo[b0 : b0 + BL],
                "vi": vi[b0 : b0 + BL].reshape(NVI, VOC),
                "neg": neg[b0 : b0 + BL].reshape(NNEG, VOC),
                "w": w,
                "bases": bases,
                "svi": svi,
                "bvo": bvo,
                "nsel": nsel,
            }
        )
    return in_maps


def kernel(vo, vi, neg_samples, V, U):
    from concourse.bass_utils import run_bass_kernel_spmd

    nc = _get_nc()
    in_maps = _make_in_maps(vo, vi, neg_samples, V, U)
    res = run_bass_kernel_spmd(nc, in_maps, core_ids=list(range(NCORES)))
    total = np.float64(0.0)
    for core in range(NCORES):
        total += np.sum(res.results[core]["out"].astype(np.float64))
    return np.float32(total / B)


# revision 29
# speedup vs baseline: 1.1185x; 1.1185x over previous
"""CBOW negative-sampling loss kernel for 8 TRN2 NeuronCores.

Strategy: data-parallel over batch (32 batch elems / core). The one-hot
inputs (1 GB total) are streamed once through SBUF; a single fused DVE
tensor_tensor_reduce per [128, F] chunk multiplies by a local iota and
reduce-adds, recovering each row's one-hot position in one pass. A tiny
cleanup converts per-chunk partial indices to global rows of W = [V; U],
gathered via indirect DMA (608 rows x 512 B / core). The remaining math
([608, 128] embeddings -> 32 per-batch losses) runs on PE (selection-matrix
matmuls for the context mean, vo broadcast, and k-sums) and ACT (Softplus).
Host sums the 8x32 per-batch losses and divides by 256.
"""

import numpy as np

VOC, DIM = 50000, 128
B, CTX, K = 256, 8, 10
NCORES = 8
BL = B // NCORES            # 32 batch elements per core
F = 5000                    # vocab chunk (free dim) per DVE op
NCH = VOC // F              # 10 chunks
NVI = BL * CTX              # 256 vi rows
NNEG = BL * K               # 320 neg rows
ROWS = BL + NVI + NNEG      # 608 one-hot rows per core
NG = (ROWS + 127) // 128    # 5 row groups
PG = [min(128, ROWS - g * 128) for g in range(NG)]  # [128,128,128,128,96]

_F32 = None  # set lazily (mybir.dt.float32)


def _row_kind(r):
    """Row r of the per-core row space -> (kind, batch, sub)."""
    if r < BL:
        return ("vo", r, 0)
    if r < BL + NVI:
        f = r - BL
        return ("vi", f // CTX, f % CTX)
    n = r - BL - NVI
    return ("neg", n // K, n % K)


def _make_constants():
    """Host-side constant tables shared by all cores."""
    bases = np.zeros((NG * 128, NCH), np.float32)
    for g in range(NG):
        for p in range(PG[g]):
            kind, _, _ = _row_kind(g * 128 + p)
            off = 0.0 if kind == "vo" else float(VOC)
            for c in range(NCH):
                bases[g * 128 + p, c] = c * F - 1 + off

    svi = np.zeros((3 * 128, BL), np.float32)     # groups 0,1,2: vi mean (1/CTX)
    for gi, g in enumerate((0, 1, 2)):
        for p in range(PG[g]):
            kind, b, _ = _row_kind(g * 128 + p)
            if kind == "vi":
                svi[gi * 128 + p, b] = 1.0 / CTX

    bvo = np.zeros((3 * BL, 128), np.float32)     # groups 2,3,4: vo broadcast
    nsel = np.zeros((3 * 128, BL), np.float32)    # groups 2,3,4: k-sum
    for gi, g in enumerate((2, 3, 4)):
        for m in range(PG[g]):
            kind, b, _ = _row_kind(g * 128 + m)
            if kind == "neg":
                bvo[gi * BL + b, m] = 1.0
                nsel[gi * 128 + m, b] = 1.0
    return bases, svi, bvo, nsel


def _split_excess_waits(nc, max_waits=1):
    """walrus CoreV3 in this container rejects instructions carrying more
    than one sync-wait; hoist extras onto standalone NoOps placed before."""
    import concourse.mybir as mybir

    uid = 0
    for f in nc.m.functions:
        for blk in f.blocks:
            insts = blk.instructions
            new_insts = []
            changed = False
            for inst in insts:
                si = inst.sync_info
                if si is not None and si.on_wait and len(si.on_wait) > max_waits:
                    waits = list(si.on_wait)
                    extra, keep = waits[:-max_waits], waits[-max_waits:]
                    for i in range(0, len(extra), max_waits):
                        nop = mybir.InstNoOp(
                            name=f"I-waitsplit-{uid}",
                            sync_info=mybir.SyncInfo(
                                on_wait=extra[i : i + max_waits], on_update=[]
                            ),
                            bass_nofuse=True,
                            engine=inst.engine,
                        )
                        uid += 1
                        new_insts.append(nop)
                    inst.sync_info = mybir.SyncInfo(
                        on_wait=keep, on_update=list(si.on_update or [])
                    )
                    changed = True
                new_insts.append(inst)
            if changed:
                blk.instructions = new_insts


def _group_srcs(vo_h, vi_h, neg_h, g, c):
    """DMA sources filling row-group g's [128, F] tile for vocab chunk c."""
    sl = slice(c * F, (c + 1) * F)
    if g == 0:
        return [(0, 32, vo_h[0:32, sl]), (32, 128, vi_h[0:96, sl])]
    if g == 1:
        return [(0, 128, vi_h[96:224, sl])]
    if g == 2:
        return [(0, 32, vi_h[224:256, sl]), (32, 128, neg_h[0:96, sl])]
    if g == 3:
        return [(0, 128, neg_h[96:224, sl])]
    return [(0, 96, neg_h[224:320, sl])]


def _build_nc(oh_bufs=4, scr_bufs=2, reps=1, dma_eng="gpsimd"):
    from contextlib import ExitStack

    import concourse.bass as bass
    import concourse.mybir as mybir
    import concourse.tile as tile

    f32 = mybir.dt.float32
    i32 = mybir.dt.int32
    Alu = mybir.AluOpType
    Act = mybir.ActivationFunctionType

    nc = bass.Bass("TRN2", target_bir_lowering=False, debug=False)

    vo_h = nc.declare_dram_parameter("vo", [BL, VOC], f32, isOutput=False)
    vi_h = nc.declare_dram_parameter("vi", [NVI, VOC], f32, isOutput=False)
    neg_h = nc.declare_dram_parameter("neg", [NNEG, VOC], f32, isOutput=False)
    w_h = nc.declare_dram_parameter("w", [2 * VOC, DIM], f32, isOutput=False)
    bases_h = nc.declare_dram_parameter("bases", [NG * 128, NCH], f32, isOutput=False)
    svi_h = nc.declare_dram_parameter("svi", [3 * 128, BL], f32, isOutput=False)
    bvo_h = nc.declare_dram_parameter("bvo", [3 * BL, 128], f32, isOutput=False)
    nsel_h = nc.declare_dram_parameter("nsel", [3 * 128, BL], f32, isOutput=False)
    out_h = nc.declare_dram_parameter("out", [BL, 1], f32, isOutput=True)

    with ExitStack() as ctx:
        tc = ctx.enter_context(tile.TileContext(nc))
        cpool = ctx.enter_context(tc.tile_pool(name="consts", bufs=1))
        ohp = ctx.enter_context(tc.tile_pool(name="oh", bufs=oh_bufs))
        scrp = ctx.enter_context(tc.tile_pool(name="scr", bufs=scr_bufs))
        accp = ctx.enter_context(tc.tile_pool(name="acc", bufs=1))
        small = ctx.enter_context(tc.tile_pool(name="small", bufs=1))
        psum = ctx.enter_context(tc.tile_pool(name="psum", bufs=1, space="PSUM"))

        iota_t = cpool.tile([128, F], f32, tag="iota")
        nc.gpsimd.iota(
            iota_t[:],
            pattern=[[1, F]],
            base=1,
            channel_multiplier=0,
            allow_small_or_imprecise_dtypes=True,
        )

        # touch Exp/Ln so their shared ACT table loads during streaming,
        # not on the tail critical path
        warm = cpool.tile([1, 1], f32, tag="warm")
        nc.vector.tensor_copy(warm[:, :], iota_t[0:1, 0:1])
        nc.scalar.activation(warm[:, :], warm[:, :], Act.Exp)
        nc.scalar.activation(warm[:, :], warm[:, :], Act.Ln, bias=1.0)

        bases_t = []
        for g in range(NG):
            t = cpool.tile([128, NCH], f32, tag=f"base{g}")
            nc.gpsimd.dma_start(
                out=t[: PG[g], :], in_=bases_h[g * 128 : g * 128 + PG[g], :]
            )
            bases_t.append(t)
        svi_t = []
        for gi in range(3):
            t = cpool.tile([128, BL], f32, tag=f"svi{gi}")
            nc.gpsimd.dma_start(out=t[:, :], in_=svi_h[gi * 128 : (gi + 1) * 128, :])
            svi_t.append(t)
        bvo_t = []
        nsel_t = []
        for gi, g in enumerate((2, 3, 4)):
            t = cpool.tile([BL, 128], f32, tag=f"bvo{gi}")
            nc.gpsimd.dma_start(out=t[:, :], in_=bvo_h[gi * BL : (gi + 1) * BL, :])
            bvo_t.append(t)
            t = cpool.tile([128, BL], f32, tag=f"nsel{gi}")
            nc.gpsimd.dma_start(
                out=t[: PG[g], :], in_=nsel_h[gi * 128 : gi * 128 + PG[g], :]
            )
            nsel_t.append(t)

        # ---- heavy phase: stream one-hot, fused mul+reduce against iota ----
        for rep in range(reps):
            _emit_body(
                nc, tc, mybir, bass, ohp, scrp, accp, small, psum,
                iota_t, bases_t, svi_t, bvo_t, nsel_t,
                vo_h, vi_h, neg_h, w_h, out_h, rep, dma_eng,
            )

    _split_excess_waits(nc)
    return nc


def _emit_body(
    nc, tc, mybir, bass, ohp, scrp, accp, small, psum,
    iota_t, bases_t, svi_t, bvo_t, nsel_t,
    vo_h, vi_h, neg_h, w_h, out_h, rep, dma_eng="gpsimd",
):
    f32 = mybir.dt.float32
    i32 = mybir.dt.int32
    Alu = mybir.AluOpType
    Act = mybir.ActivationFunctionType
    if dma_eng == "alt":
        dmas = [nc.sync, nc.gpsimd]
    elif dma_eng == "alt2hw":
        dmas = [nc.sync, nc.scalar]
    else:
        dmas = [getattr(nc, dma_eng)]
    if True:
        acc_t = [
            accp.tile([128, NCH], f32, name=f"acc{g}", tag=f"acc{g}")
            for g in range(NG)
        ]
        emb_t = []
        for g in range(NG):
            p = PG[g]
            for c in range(NCH):
                oh = ohp.tile([128, F], f32, tag="oh")
                for lo, hi, src in _group_srcs(vo_h, vi_h, neg_h, g, c):
                    dmas[(g * NCH + c) % len(dmas)].dma_start(
                        out=oh[lo:hi, :], in_=src
                    )
                scr = scrp.tile([128, F], f32, tag="scr")
                nc.vector.tensor_mul(scr[:p, :], oh[:p, :], iota_t[:p, :])
                nc.scalar.activation(
                    out=scr[:p, :],
                    in_=scr[:p, :],
                    func=Act.Identity,
                    accum_out=acc_t[g][:p, c : c + 1],
                )

            # ---- per-group cleanup: partial sums -> global W row index ----
            hit = small.tile([128, NCH], f32, tag=f"hit{g}")
            nc.vector.tensor_scalar_min(hit[:p, :], acc_t[g][:p, :], 1.0)
            t1 = small.tile([128, NCH], f32, tag=f"t1{g}")
            nc.vector.tensor_mul(t1[:p, :], hit[:p, :], bases_t[g][:p, :])
            t2 = small.tile([128, NCH], f32, tag=f"t2{g}")
            nc.vector.tensor_add(t2[:p, :], acc_t[g][:p, :], t1[:p, :])
            idxf = small.tile([128, 1], f32, tag=f"idxf{g}")
            nc.vector.tensor_reduce(
                idxf[:p, :], t2[:p, :], axis=mybir.AxisListType.X, op=Alu.add
            )
            # clamp in-bounds (safety; also keeps rep>0 timing runs legal)
            nc.vector.tensor_scalar_min(idxf[:p, :], idxf[:p, :], float(2 * VOC - 1))
            idxi = small.tile([128, 1], i32, tag=f"idxi{g}")
            nc.vector.tensor_copy(idxi[:p, :], idxf[:p, :])

            emb = small.tile([128, DIM], f32, tag=f"emb{g}")
            nc.gpsimd.indirect_dma_start(
                out=emb[:p, :],
                out_offset=None,
                in_=w_h[:, :],
                in_offset=bass.IndirectOffsetOnAxis(ap=idxi[:p, 0:1], axis=0),
            )
            emb_t.append(emb)

        # ---- tail: vi mean, dots, softplus, k-sums -> per-batch loss ----
        vi_ps = psum.tile([BL, DIM], f32, tag="vi_ps")
        for gi, g in enumerate((0, 1, 2)):
            nc.tensor.matmul(
                out=vi_ps[:, :],
                lhsT=svi_t[gi][: PG[g], :],
                rhs=emb_t[g][: PG[g], :],
                start=(gi == 0),
                stop=(gi == 2),
            )

        vo_emb = emb_t[0][0:BL, :]
        scrd = small.tile([BL, DIM], f32, tag="scrd")
        dotvv = small.tile([BL, 1], f32, tag="dotvv")
        nc.vector.tensor_mul(scrd[:, :], vo_emb, vi_ps[:, :])
        nc.vector.tensor_reduce(
            dotvv[:, :], scrd[:, :], axis=mybir.AxisListType.X, op=Alu.add
        )
        # softplus(x) = ln(1 + exp(x)); ln & exp share one ACT table set here
        nl_e = small.tile([BL, 1], f32, tag="nl_e")
        nc.scalar.activation(nl_e[:, :], dotvv[:, :], Act.Exp, scale=-1.0)
        nl = small.tile([BL, 1], f32, tag="nl")
        nc.scalar.activation(nl[:, :], nl_e[:, :], Act.Ln, bias=1.0)

        right_ps = psum.tile([BL, 1], f32, tag="right_ps")
        for gi, g in enumerate((2, 3, 4)):
            p = PG[g]
            bc = psum.tile([128, DIM], f32, tag=f"bc{gi}")
            nc.tensor.matmul(
                out=bc[:p, :],
                lhsT=bvo_t[gi][:, :p],
                rhs=vo_emb,
                start=True,
                stop=True,
            )
            scr2 = small.tile([128, DIM], f32, tag=f"scr2{gi}")
            nd = small.tile([128, 1], f32, tag=f"nd{gi}")
            nc.vector.tensor_mul(scr2[:p, :], emb_t[g][:p, :], bc[:p, :])
            nc.vector.tensor_reduce(
                nd[:p, :], scr2[:p, :], axis=mybir.AxisListType.X, op=Alu.add
            )
            sp_e = small.tile([128, 1], f32, tag=f"sp_e{gi}")
            nc.scalar.activation(sp_e[:p, :], nd[:p, :], Act.Exp)
            sp = small.tile([128, 1], f32, tag=f"sp{gi}")
            nc.scalar.activation(sp[:p, :], sp_e[:p, :], Act.Ln, bias=1.0)
            nc.tensor.matmul(
                out=right_ps[:, :],
                lhsT=nsel_t[gi][:p, :],
                rhs=sp[:p, :],
                start=(gi == 0),
                stop=(gi == 2),
            )

        loss = small.tile([BL, 1], f32, tag="loss")
        nc.vector.tensor_add(loss[:, :], nl[:, :], right_ps[:, :])
        nc.gpsimd.dma_start(out=out_h[:, :], in_=loss[:, :])


_BUILT = {}


def _get_nc(key=(4, 2, 1, "alt")):
    if key not in _BUILT:
        _BUILT[key] = _build_nc(*key)
    return _BUILT[key]


def _make_in_maps(vo, vi, neg_samples, V, U):
    vo = np.ascontiguousarray(vo, np.float32)
    vi = np.ascontiguousarray(vi, np.float32).reshape(B, CTX, VOC)
    neg = np.ascontiguousarray(neg_samples, np.float32).reshape(B, K, VOC)
    w = np.ascontiguousarray(np.concatenate([V, U], axis=0), np.float32)
    bases, svi, bvo, nsel = _make_constants()
    in_maps = []
    for core in range(NCORES):
        b0 = core * BL
        in_maps.append(
            {
                "vo": vo[b0 : b0 + BL],
                "vi": vi[b0 : b0 + BL].reshape(NVI, VOC),
                "neg": neg[b0 : b0 + BL].reshape(NNEG, VOC),
                "w": w,
                "bases": bases,
                "svi": svi,
                "bvo": bvo,
                "nsel": nsel,
            }
        )
    return in_maps


def kernel(vo, vi, neg_samples, V, U):
    import time

    from concourse.bass_utils import run_bass_kernel_spmd

    in_maps = _make_in_maps(vo, vi, neg_samples, V, U)
    last_err = None
    for attempt in range(3):
        try:
            nc = _get_nc()
            res = run_bass_kernel_spmd(nc, in_maps, core_ids=list(range(NCORES)))
            break
        except Exception as e:  # transient NRT_EXEC_UNIT_UNRECOVERABLE, etc.
            last_err = e
            _BUILT.clear()  # rebuild the module on retry
            time.sleep(10.0 * (attempt + 1))
    else:
        raise last_err
    total = np.float64(0.0)
    for core in range(NCORES):
        total += np.sum(res.results[core]["out"].astype(np.float64))
    return np.float32(total / B)
